# revision 27
# baseline (speedup 1.0000x reference)
"""Depthwise 3x3 conv (center tap zeroed) + residual, on 8 trn2 cores.

Layout strategy (per core, data-parallel over batch: 4 batches/core):
  - x arrives NHWC. Rows (b,h) go on SBUF partitions, (w,c) on free dim.
  - PE transpose-in puts channels on partitions: x_T[c, (w_pad, b, h_pad)]
    (bf16, zero-padded borders) so each tap is a free-dim offset and the
    per-channel tap weight is a diagonal 128x128 stationary matrix.
  - 8 taps = 8 diagonal-stationary matmuls accumulating in PSUM (fp32).
  - PE transpose-out back to natural rows layout; DVE adds the exact-fp32
    residual (x itself == center tap with weight 1) in place.
HW does ~2x98MB of HBM IO; conv accumulation in fp32 PSUM; only the conv
term passes through bf16 (residual stays exact fp32).
"""

import sys

if "/opt/trn_rl_repo" not in sys.path:
    sys.path.insert(0, "/opt/trn_rl_repo")

from contextlib import ExitStack

import ml_dtypes
import numpy as np

import concourse.bass as bass
import concourse.tile as tile
from concourse import bacc, mybir
from concourse.bass_utils import run_bass_kernel_spmd

B, H, W, C = 32, 56, 56, 256
N_CORES = 8
BPC = B // N_CORES          # 4 batches per core
RT = 2                      # row-tiles per core
RTB = BPC // RT             # 2 batches per row-tile
ROWS = RTB * H              # 112 partition rows per tile
HP, WP = H + 2, W + 2       # padded spatial dims (58)
NHALF = C // 128            # 2 channel halves
NTAP = 8
# Taps split between engines: PE does 6 (PSUM accumulate), DVE does 2 as
# fused scalar_tensor_tensor on the evacuated bf16 tile. The DVE taps must
# have even element offsets in the padded layout (dh != 0) so the bf16
# 2x_1P perf mode engages.
PE_TAPS = [(-1, 0), (-1, 1), (0, -1), (0, 1), (1, -1), (1, 0), (1, 1)]
DVE_TAPS = [(-1, -1)]
TAPS = PE_TAPS + DVE_TAPS

F32 = mybir.dt.float32
F32R = mybir.dt.float32r
BF16 = mybir.dt.bfloat16

_CACHE = {}
LAST_EXEC_NS = None
LAST_RESULT = None


def _patch_ldw_opt():
    """Flip walrus --enable-ldw-opt to true: our conv re-loads the same
    diagonal stationary for consecutive matmuls, and un-hidden LDWEIGHTS
    costs ~48us/core of TensorE time otherwise."""
    import concourse.bass_utils as bu

    if getattr(bu, "_ldw_patched", False):
        return
    orig = bu.run_command

    def patched(argv, **kwargs):
        argv = [
            a.replace("--enable-ldw-opt=false", "--enable-ldw-opt=true")
            if isinstance(a, str)
            else a
            for a in argv
        ]
        return orig(argv, **kwargs)

    bu.run_command = patched
    bu._ldw_patched = True


# NOTE: ldw-opt=true crashes walrus codegen (visitInstLdweights) for both
# fp32 and bf16 transpose paths in this compiler build — leave it off.


def _build_nc(trace=False):
    nc = bacc.Bacc("TRN2", target_bir_lowering=False, debug=False, num_devices=N_CORES)
    x_ext = nc.dram_tensor("x", [RT, ROWS, W, C], F32, kind="ExternalInput").ap()
    wd_ext = nc.dram_tensor("wd", [128, NHALF * NTAP * 128], BF16, kind="ExternalInput").ap()
    wv_ext = nc.dram_tensor("wv", [128, NHALF * NTAP], F32, kind="ExternalInput").ap()
    idb_ext = nc.dram_tensor("idb", [128, 128], BF16, kind="ExternalInput").ap()
    idf_ext = nc.dram_tensor("idf", [128, 128], F32, kind="ExternalInput").ap()
    out_ext = nc.dram_tensor("out", [RT, ROWS, W, C], F32, kind="ExternalOutput").ap()

    with tile.TileContext(nc) as tc, ExitStack() as ctx:
        const = ctx.enter_context(tc.tile_pool(name="const", bufs=1))
        xn_pool = ctx.enter_context(tc.tile_pool(name="xn", bufs=14))
        xnb_pool = ctx.enter_context(tc.tile_pool(name="xnb", bufs=4))
        xt_pool = ctx.enter_context(tc.tile_pool(name="xt", bufs=2))
        c8_pool = ctx.enter_context(tc.tile_pool(name="c8", bufs=2))
        ps_tin = ctx.enter_context(tc.tile_pool(name="ps_tin", bufs=2, space="PSUM"))
        ps_conv = ctx.enter_context(tc.tile_pool(name="ps_conv", bufs=3, space="PSUM"))
        ps_tout = ctx.enter_context(tc.tile_pool(name="ps_tout", bufs=3, space="PSUM"))

        wd = const.tile([128, NHALF * NTAP * 128], BF16)
        nc.sync.dma_start(wd[:], wd_ext)
        idb = const.tile([128, 128], BF16)
        nc.sync.dma_start(idb[:], idb_ext)
        idf = const.tile([128, 128], F32)
        nc.sync.dma_start(idf[:], idf_ext)
        wv = const.tile([128, NHALF * NTAP], F32)
        nc.sync.dma_start(wv[:], wv_ext)

        NWG = W // 4  # 14 groups of 4 w-columns
        NW8 = W // 8  # 7 groups of 8 w-columns

        for rt in range(RT):
            # x_n split into 7 tiles of 8 w-columns: DMA, cast, residual and
            # store all operate at this granularity so nothing waits on the
            # whole 6.4MB row-tile.
            xn_tiles = []
            for og in range(NW8):
                x_n = xn_pool.tile([ROWS, 8, C], F32, name=f"x_n_{rt}_{og}", tag="x_n")
                xn_tiles.append(x_n)
                nc.sync.dma_start(
                    x_n[:], x_ext[rt, :, og * 8 : (og + 1) * 8, :]
                )

            # ---- phase A: cast to bf16, transpose-in both halves ----
            xts = []
            for hf in range(NHALF):
                x_t = xt_pool.tile(
                    [128, WP, RTB, HP], BF16, name=f"x_t_{rt}_{hf}", tag="x_t"
                )
                xts.append(x_t)
                # zero the pad borders (w=0, w=57, h=0, h=57)
                nc.gpsimd.memset(x_t[:, 0, :, :], 0.0)
                nc.gpsimd.memset(x_t[:, WP - 1, :, :], 0.0)
                nc.gpsimd.memset(x_t[:, 1 : WP - 1, :, 0], 0.0)
                nc.gpsimd.memset(x_t[:, 1 : WP - 1, :, HP - 1], 0.0)
                for og in range(NW8):
                    # cast this 8-w group's c-half to bf16 (DVE 2x)
                    x_nb = xnb_pool.tile([ROWS, 8, 128], BF16, name="x_nb")
                    nc.vector.tensor_copy(
                        x_nb[:],
                        xn_tiles[og][:, :, hf * 128 : (hf + 1) * 128],
                    )
                    for half in range(2):
                        pt = ps_tin.tile([128, 4, RTB, H], BF16, name="pt")
                        for wl in range(4):
                            nc.tensor.transpose(
                                pt[:, wl, :, :],
                                x_nb[:, half * 4 + wl, :],
                                idb[0:ROWS, 0:ROWS],
                            )
                        wg = og * 2 + half
                        nc.scalar.copy(
                            x_t[:, 1 + 4 * wg : 5 + 4 * wg, :, 1 : H + 1], pt[:]
                        )

            # ---- phase B: conv + transpose-out per half ----
            for hf in range(NHALF):
                x_t = xts[hf]
                c8t = c8_pool.tile(
                    [128, W, RTB, H], BF16, name=f"c8t_{rt}_{hf}", tag="c8t"
                )
                for cg in range(NWG):
                    pc = ps_conv.tile([128, 4, RTB, H], F32, name="pc")
                    w0 = 1 + 4 * cg
                    for t, (dh, dw) in enumerate(PE_TAPS):
                        mov = x_t[
                            :, w0 + dw : w0 + 4 + dw, :, 1 + dh : H + 1 + dh
                        ]
                        nc.tensor.matmul(
                            pc[:],
                            wd[:, (hf * NTAP + t) * 128 : (hf * NTAP + t + 1) * 128],
                            mov,
                            start=(t == 0),
                            stop=(t == len(PE_TAPS) - 1),
                        )
                    nc.scalar.copy(c8t[:, 4 * cg : 4 * cg + 4, :, :], pc[:])

                # DVE tap over 8-w groups (merged: fewer, larger ops)
                for j, (dh, dw) in enumerate(DVE_TAPS):
                    t = len(PE_TAPS) + j
                    for og in range(NW8):
                        w0 = 1 + 8 * og
                        dst = c8t[:, 8 * og : 8 * og + 8, :, :]
                        mov = x_t[
                            :, w0 + dw : w0 + 8 + dw, :, 1 + dh : H + 1 + dh
                        ]
                        nc.vector.scalar_tensor_tensor(
                            dst,
                            mov,
                            wv[:, hf * NTAP + t : hf * NTAP + t + 1],
                            dst,
                            mybir.AluOpType.mult,
                            mybir.AluOpType.add,
                        )

                for og in range(NW8):
                    po = ps_tout.tile([ROWS, 8, 128], BF16, name="po")
                    for wl in range(8):
                        w = og * 8 + wl
                        nc.tensor.transpose(po[:, wl, :], c8t[:, w, :, :], idb[:, :])
                    dst = xn_tiles[og][:, :, hf * 128 : (hf + 1) * 128]
                    nc.vector.tensor_add(dst, po[:], dst)

            for og in range(NW8):
                nc.sync.dma_start(
                    out_ext[rt, :, og * 8 : (og + 1) * 8, :], xn_tiles[og][:]
                )

    nc.compile()
    return nc


def _weights_np(kernel):
    """kernel: [3,3,C] f32 -> diag stationaries [128, NHALF*NTAP*128] bf16."""
    k = np.asarray(kernel, dtype=np.float32)
    wd = np.zeros((128, NHALF, NTAP, 128), dtype=np.float32)
    for hf in range(NHALF):
        for t, (dh, dw) in enumerate(TAPS):
            wd[np.arange(128), hf, t, np.arange(128)] = k[
                dh + 1, dw + 1, hf * 128 : (hf + 1) * 128
            ]
    return wd.reshape(128, NHALF * NTAP * 128).astype(ml_dtypes.bfloat16)


def _weights_vec_np(kernel):
    k = np.asarray(kernel, dtype=np.float32)
    wv = np.zeros((128, NHALF, NTAP), dtype=np.float32)
    for hf in range(NHALF):
        for t, (dh, dw) in enumerate(TAPS):
            wv[:, hf, t] = k[dh + 1, dw + 1, hf * 128 : (hf + 1) * 128]
    return wv.reshape(128, NHALF * NTAP)


def _install_ntff_hook():
    """The container's antenv lacks axon_hooks; rebuild the NTFF profile hook
    via ctypes against the injected libaxon_pjrt.so (same ABI as trn_boot)."""
    import contextlib
    import ctypes
    import types

    try:
        from antenv.axon_hooks import get_axon_ntff_profile_hook  # noqa: F401

        return
    except ImportError:
        pass
    so = "/opt/axon/libaxon_pjrt.so"
    try:
        lib = ctypes.CDLL(so)
    except OSError:
        return
    if not hasattr(lib, "axon_start_nrt_profile"):
        return
    lib.axon_start_nrt_profile.argtypes = [
        ctypes.POINTER(ctypes.c_int64),
        ctypes.c_size_t,
    ]
    lib.axon_start_nrt_profile.restype = ctypes.c_int64
    lib.axon_stop_nrt_profile.argtypes = [ctypes.c_char_p]
    lib.axon_stop_nrt_profile.restype = ctypes.c_int64

    @contextlib.contextmanager
    def _hook(output_dir, device_ids):
        import jax

        jax.devices()
        if device_ids:
            ids = (ctypes.c_int64 * len(device_ids))(*device_ids)
            rc = lib.axon_start_nrt_profile(ids, len(device_ids))
        else:
            rc = lib.axon_start_nrt_profile(None, 0)
        if rc != 0:
            raise RuntimeError(f"axon_start_nrt_profile rc={rc}")
        try:
            yield
        finally:
            n = lib.axon_stop_nrt_profile(str(output_dir).encode())
            print(f"profile: {n} ntff file(s) -> {output_dir}")

    mod = types.ModuleType("antenv.axon_hooks")
    mod.set_axon_ntff_profile_hook = lambda h: None
    mod.get_axon_ntff_profile_hook = lambda: _hook
    sys.modules["antenv.axon_hooks"] = mod
    # avoid the network artifact upload in the trace path
    import concourse.bass_utils as bu

    bu.upload_artifacts = lambda tmpdir: tmpdir


def kernel(x, kernel):
    global LAST_EXEC_NS, LAST_RESULT
    x = np.ascontiguousarray(np.asarray(x, dtype=np.float32))
    assert x.shape == (B, H, W, C)

    if "nc" not in _CACHE:
        _CACHE["nc"] = _build_nc()
    nc = _CACHE["nc"]

    wd = _weights_np(kernel)
    idb = np.eye(128, dtype=np.float32).astype(ml_dtypes.bfloat16)
    idf = np.eye(128, dtype=np.float32)

    wv = _weights_vec_np(kernel)
    xs = x.reshape(N_CORES, RT, ROWS, W, C)
    in_maps = [
        {"x": xs[i], "wd": wd, "idb": idb, "idf": idf, "wv": wv}
        for i in range(N_CORES)
    ]

    import os

    trace = bool(int(os.environ.get("KERNEL_TRACE", "0")))
    tmpdir = None
    if trace:
        _install_ntff_hook()
        tmpdir = os.environ.get("KERNEL_TRACE_DIR") or None
    res = run_bass_kernel_spmd(
        nc, in_maps, list(range(N_CORES)), trace=trace, tmpdir=tmpdir
    )
    LAST_RESULT = res
    LAST_EXEC_NS = res.exec_time_ns

    out = np.empty((N_CORES, RT, ROWS, W, C), dtype=np.float32)
    for i in range(N_CORES):
        out[i] = res.results[i]["out"]
    return out.reshape(B, H, W, C)


# revision 29
# speedup vs baseline: 1.0129x; 1.0129x over previous
"""Depthwise 3x3 conv (center tap zeroed) + residual, on 8 trn2 cores.

Layout strategy (per core, data-parallel over batch: 4 batches/core):
  - x arrives NHWC. Rows (b,h) go on SBUF partitions, (w,c) on free dim.
  - PE transpose-in puts channels on partitions: x_T[c, (w_pad, b, h_pad)]
    (bf16, zero-padded borders) so each tap is a free-dim offset and the
    per-channel tap weight is a diagonal 128x128 stationary matrix.
  - 8 taps = 8 diagonal-stationary matmuls accumulating in PSUM (fp32).
  - PE transpose-out back to natural rows layout; DVE adds the exact-fp32
    residual (x itself == center tap with weight 1) in place.
HW does ~2x98MB of HBM IO; conv accumulation in fp32 PSUM; only the conv
term passes through bf16 (residual stays exact fp32).
"""

import sys

if "/opt/trn_rl_repo" not in sys.path:
    sys.path.insert(0, "/opt/trn_rl_repo")

from contextlib import ExitStack

import ml_dtypes
import numpy as np

import concourse.bass as bass
import concourse.tile as tile
from concourse import bacc, mybir
from concourse.bass_utils import run_bass_kernel_spmd

B, H, W, C = 32, 56, 56, 256
N_CORES = 8
BPC = B // N_CORES          # 4 batches per core
RT = 2                      # row-tiles per core
RTB = BPC // RT             # 2 batches per row-tile
ROWS = RTB * H              # 112 partition rows per tile
HP, WP = H + 2, W + 2       # padded spatial dims (58)
NHALF = C // 128            # 2 channel halves
NTAP = 8
# Taps split between engines: PE does 6 (PSUM accumulate), DVE does 2 as
# fused scalar_tensor_tensor on the evacuated bf16 tile. The DVE taps must
# have even element offsets in the padded layout (dh != 0) so the bf16
# 2x_1P perf mode engages.
PE_TAPS = [(-1, 0), (-1, 1), (0, -1), (0, 1), (1, -1), (1, 0), (1, 1)]
DVE_TAPS = [(-1, -1)]
TAPS = PE_TAPS + DVE_TAPS

F32 = mybir.dt.float32
F32R = mybir.dt.float32r
BF16 = mybir.dt.bfloat16

_CACHE = {}
LAST_EXEC_NS = None
LAST_RESULT = None


def _patch_ldw_opt():
    """Flip walrus --enable-ldw-opt to true: our conv re-loads the same
    diagonal stationary for consecutive matmuls, and un-hidden LDWEIGHTS
    costs ~48us/core of TensorE time otherwise."""
    import concourse.bass_utils as bu

    if getattr(bu, "_ldw_patched", False):
        return
    orig = bu.run_command

    def patched(argv, **kwargs):
        argv = [
            a.replace("--enable-ldw-opt=false", "--enable-ldw-opt=true")
            if isinstance(a, str)
            else a
            for a in argv
        ]
        return orig(argv, **kwargs)

    bu.run_command = patched
    bu._ldw_patched = True


# NOTE: ldw-opt=true crashes walrus codegen (visitInstLdweights) for both
# fp32 and bf16 transpose paths in this compiler build — leave it off.


def _build_nc(trace=False):
    nc = bacc.Bacc("TRN2", target_bir_lowering=False, debug=False, num_devices=N_CORES)
    x_ext = nc.dram_tensor("x", [RT, ROWS, W, C], F32, kind="ExternalInput").ap()
    wd_ext = nc.dram_tensor("wd", [128, NHALF * NTAP * 128], BF16, kind="ExternalInput").ap()
    wv_ext = nc.dram_tensor("wv", [128, NHALF * NTAP], F32, kind="ExternalInput").ap()
    idb_ext = nc.dram_tensor("idb", [128, 128], BF16, kind="ExternalInput").ap()
    idf_ext = nc.dram_tensor("idf", [128, 128], F32, kind="ExternalInput").ap()
    out_ext = nc.dram_tensor("out", [RT, ROWS, W, C], F32, kind="ExternalOutput").ap()

    with tile.TileContext(nc) as tc, ExitStack() as ctx:
        const = ctx.enter_context(tc.tile_pool(name="const", bufs=1))
        xn_pool = ctx.enter_context(tc.tile_pool(name="xn", bufs=14))
        xnb_pool = ctx.enter_context(tc.tile_pool(name="xnb", bufs=4))
        xt_pool = ctx.enter_context(tc.tile_pool(name="xt", bufs=2))
        c8_pool = ctx.enter_context(tc.tile_pool(name="c8", bufs=2))
        ps_tin = ctx.enter_context(tc.tile_pool(name="ps_tin", bufs=2, space="PSUM"))
        ps_conv = ctx.enter_context(tc.tile_pool(name="ps_conv", bufs=4, space="PSUM"))
        ps_tout = ctx.enter_context(tc.tile_pool(name="ps_tout", bufs=2, space="PSUM"))

        wd = const.tile([128, NHALF * NTAP * 128], BF16)
        nc.sync.dma_start(wd[:], wd_ext)
        idb = const.tile([128, 128], BF16)
        nc.sync.dma_start(idb[:], idb_ext)
        idf = const.tile([128, 128], F32)
        nc.sync.dma_start(idf[:], idf_ext)
        wv = const.tile([128, NHALF * NTAP], F32)
        nc.sync.dma_start(wv[:], wv_ext)

        NWG = W // 4  # 14 groups of 4 w-columns
        NW8 = W // 8  # 7 groups of 8 w-columns

        for rt in range(RT):
            # x_n split into 7 tiles of 8 w-columns: DMA, cast, residual and
            # store all operate at this granularity so nothing waits on the
            # whole 6.4MB row-tile.
            xn_tiles = []
            for og in range(NW8):
                x_n = xn_pool.tile([ROWS, 8, C], F32, name=f"x_n_{rt}_{og}", tag="x_n")
                xn_tiles.append(x_n)
                nc.sync.dma_start(
                    x_n[:], x_ext[rt, :, og * 8 : (og + 1) * 8, :]
                )

            # ---- phase A: cast to bf16, transpose-in both halves ----
            xts = []
            for hf in range(NHALF):
                x_t = xt_pool.tile(
                    [128, WP, RTB, HP], BF16, name=f"x_t_{rt}_{hf}", tag="x_t"
                )
                xts.append(x_t)
                # zero the pad borders (w=0, w=57, h=0, h=57)
                nc.gpsimd.memset(x_t[:, 0, :, :], 0.0)
                nc.gpsimd.memset(x_t[:, WP - 1, :, :], 0.0)
                nc.gpsimd.memset(x_t[:, 1 : WP - 1, :, 0], 0.0)
                nc.gpsimd.memset(x_t[:, 1 : WP - 1, :, HP - 1], 0.0)
                for og in range(NW8):
                    # cast this 8-w group's c-half to bf16 (DVE 2x)
                    x_nb = xnb_pool.tile([ROWS, 8, 128], BF16, name="x_nb")
                    nc.vector.tensor_copy(
                        x_nb[:],
                        xn_tiles[og][:, :, hf * 128 : (hf + 1) * 128],
                    )
                    pt = ps_tin.tile([128, 8, RTB, H], BF16, name="pt")
                    for wl in range(8):
                        nc.tensor.transpose(
                            pt[:, wl, :, :],
                            x_nb[:, wl, :],
                            idb[0:ROWS, 0:ROWS],
                        )
                    nc.scalar.copy(
                        x_t[:, 1 + 8 * og : 9 + 8 * og, :, 1 : H + 1], pt[:]
                    )

            # ---- phase B: conv + transpose-out per half ----
            for hf in range(NHALF):
                x_t = xts[hf]
                c8t = c8_pool.tile(
                    [128, W, RTB, H], BF16, name=f"c8t_{rt}_{hf}", tag="c8t"
                )
                for cg in range(NWG):
                    pc = ps_conv.tile([128, 4, RTB, H], F32, name="pc")
                    w0 = 1 + 4 * cg
                    for t, (dh, dw) in enumerate(PE_TAPS):
                        mov = x_t[
                            :, w0 + dw : w0 + 4 + dw, :, 1 + dh : H + 1 + dh
                        ]
                        nc.tensor.matmul(
                            pc[:],
                            wd[:, (hf * NTAP + t) * 128 : (hf * NTAP + t + 1) * 128],
                            mov,
                            start=(t == 0),
                            stop=(t == len(PE_TAPS) - 1),
                        )
                    nc.scalar.copy(c8t[:, 4 * cg : 4 * cg + 4, :, :], pc[:])

                # DVE tap over 8-w groups (merged: fewer, larger ops)
                for j, (dh, dw) in enumerate(DVE_TAPS):
                    t = len(PE_TAPS) + j
                    for og in range(NW8):
                        w0 = 1 + 8 * og
                        dst = c8t[:, 8 * og : 8 * og + 8, :, :]
                        mov = x_t[
                            :, w0 + dw : w0 + 8 + dw, :, 1 + dh : H + 1 + dh
                        ]
                        nc.vector.scalar_tensor_tensor(
                            dst,
                            mov,
                            wv[:, hf * NTAP + t : hf * NTAP + t + 1],
                            dst,
                            mybir.AluOpType.mult,
                            mybir.AluOpType.add,
                        )

                for og in range(NW8):
                    po = ps_tout.tile([ROWS, 8, 128], BF16, name="po")
                    for wl in range(8):
                        w = og * 8 + wl
                        nc.tensor.transpose(po[:, wl, :], c8t[:, w, :, :], idb[:, :])
                    dst = xn_tiles[og][:, :, hf * 128 : (hf + 1) * 128]
                    nc.vector.tensor_add(dst, po[:], dst)

            for og in range(NW8):
                nc.sync.dma_start(
                    out_ext[rt, :, og * 8 : (og + 1) * 8, :], xn_tiles[og][:]
                )

    nc.compile()
    return nc


def _weights_np(kernel):
    """kernel: [3,3,C] f32 -> diag stationaries [128, NHALF*NTAP*128] bf16."""
    k = np.asarray(kernel, dtype=np.float32)
    wd = np.zeros((128, NHALF, NTAP, 128), dtype=np.float32)
    for hf in range(NHALF):
        for t, (dh, dw) in enumerate(TAPS):
            wd[np.arange(128), hf, t, np.arange(128)] = k[
                dh + 1, dw + 1, hf * 128 : (hf + 1) * 128
            ]
    return wd.reshape(128, NHALF * NTAP * 128).astype(ml_dtypes.bfloat16)


def _weights_vec_np(kernel):
    k = np.asarray(kernel, dtype=np.float32)
    wv = np.zeros((128, NHALF, NTAP), dtype=np.float32)
    for hf in range(NHALF):
        for t, (dh, dw) in enumerate(TAPS):
            wv[:, hf, t] = k[dh + 1, dw + 1, hf * 128 : (hf + 1) * 128]
    return wv.reshape(128, NHALF * NTAP)


def _install_ntff_hook():
    """The container's antenv lacks axon_hooks; rebuild the NTFF profile hook
    via ctypes against the injected libaxon_pjrt.so (same ABI as trn_boot)."""
    import contextlib
    import ctypes
    import types

    try:
        from antenv.axon_hooks import get_axon_ntff_profile_hook  # noqa: F401

        return
    except ImportError:
        pass
    so = "/opt/axon/libaxon_pjrt.so"
    try:
        lib = ctypes.CDLL(so)
    except OSError:
        return
    if not hasattr(lib, "axon_start_nrt_profile"):
        return
    lib.axon_start_nrt_profile.argtypes = [
        ctypes.POINTER(ctypes.c_int64),
        ctypes.c_size_t,
    ]
    lib.axon_start_nrt_profile.restype = ctypes.c_int64
    lib.axon_stop_nrt_profile.argtypes = [ctypes.c_char_p]
    lib.axon_stop_nrt_profile.restype = ctypes.c_int64

    @contextlib.contextmanager
    def _hook(output_dir, device_ids):
        import jax

        jax.devices()
        if device_ids:
            ids = (ctypes.c_int64 * len(device_ids))(*device_ids)
            rc = lib.axon_start_nrt_profile(ids, len(device_ids))
        else:
            rc = lib.axon_start_nrt_profile(None, 0)
        if rc != 0:
            raise RuntimeError(f"axon_start_nrt_profile rc={rc}")
        try:
            yield
        finally:
            n = lib.axon_stop_nrt_profile(str(output_dir).encode())
            print(f"profile: {n} ntff file(s) -> {output_dir}")

    mod = types.ModuleType("antenv.axon_hooks")
    mod.set_axon_ntff_profile_hook = lambda h: None
    mod.get_axon_ntff_profile_hook = lambda: _hook
    sys.modules["antenv.axon_hooks"] = mod
    # avoid the network artifact upload in the trace path
    import concourse.bass_utils as bu

    bu.upload_artifacts = lambda tmpdir: tmpdir


def kernel(x, kernel):
    global LAST_EXEC_NS, LAST_RESULT
    x = np.ascontiguousarray(np.asarray(x, dtype=np.float32))
    assert x.shape == (B, H, W, C)

    if "nc" not in _CACHE:
        _CACHE["nc"] = _build_nc()
    nc = _CACHE["nc"]

    wd = _weights_np(kernel)
    idb = np.eye(128, dtype=np.float32).astype(ml_dtypes.bfloat16)
    idf = np.eye(128, dtype=np.float32)

    wv = _weights_vec_np(kernel)
    xs = x.reshape(N_CORES, RT, ROWS, W, C)
    in_maps = [
        {"x": xs[i], "wd": wd, "idb": idb, "idf": idf, "wv": wv}
        for i in range(N_CORES)
    ]

    import os

    trace = bool(int(os.environ.get("KERNEL_TRACE", "0")))
    tmpdir = None
    if trace:
        _install_ntff_hook()
        tmpdir = os.environ.get("KERNEL_TRACE_DIR") or None
    res = run_bass_kernel_spmd(
        nc, in_maps, list(range(N_CORES)), trace=trace, tmpdir=tmpdir
    )
    LAST_RESULT = res
    LAST_EXEC_NS = res.exec_time_ns

    out = np.empty((N_CORES, RT, ROWS, W, C), dtype=np.float32)
    for i in range(N_CORES):
        out[i] = res.results[i]["out"]
    return out.reshape(B, H, W, C)
